# revision 29
# baseline (speedup 1.0000x reference)
"""GRU Seq2Seq Trainium2 kernel (nn_GRU_Seq2Seq_83219286327778).

Strategy: data-parallel over batch (2048 -> 8 x 256), gate-major transposed
layout on-device ([hidden/gate dim on partitions, batch on free dim]) so the
recurrence needs no transposes.  gx+gh accumulate in PSUM; gate biases are
applied via ACT-engine bias operands (no rank-1 bias matmuls).  Per cell the
matmuls whose operand is already available (recurrent gh for L1/decoder
cells) are emitted first so the PE stays busy across every activation tail
(keeps HAM at full clock).  The decoder fc4 feedback dW0@W4 is rank-64 and
runs two-stage: u = [W4;W1]@h1 (the W1 row doubles as the out1 readout),
then gx = dW0@u.  Matmuls + elementwise run in bf16 (fp32 PSUM accumulate).
"""
import sys
sys.path.insert(0, "/opt/trn_rl_repo")
import numpy as np

B, LAGS, HORIZONS, F, H = 2048, 64, 24, 64, 512
NCORES = 8
BL = B // NCORES           # 256 batch per core
G3 = 3 * H                 # 1536
KC = H // 128              # 4 k-chunks
SRC_CHUNK = 8              # timesteps per src DMA

_RUNNER = None


def _build_nc(mm_dt_name="bfloat16", repeat=1, lags=LAGS, horizons=HORIZONS, dump_h=False):
    import concourse.tile as tile
    from concourse import mybir, bacc

    F32 = mybir.dt.float32
    MMD = getattr(mybir.dt, mm_dt_name)
    AF = mybir.ActivationFunctionType
    OP = mybir.AluOpType

    nc = bacc.Bacc("TRN2", target_bir_lowering=False)

    srcT_d = nc.dram_tensor("srcT", [F + 1, LAGS, BL], MMD, kind="ExternalInput")
    wnames = ["eu0", "ew1", "eu1", "du0", "dw1", "du1"]
    w_d = {n: nc.dram_tensor(n, [H, G3], MMD, kind="ExternalInput") for n in wnames}
    ew0a_d = nc.dram_tensor("ew0a", [F + 1, G3], MMD, kind="ExternalInput")
    dw0a_d = nc.dram_tensor("dw0a", [F + 1, G3], MMD, kind="ExternalInput")
    dw0al_d = nc.dram_tensor("dw0al", [F + 1, G3], MMD, kind="ExternalInput")
    # per-partition activation biases: 5 sets (e0,e1,d0first,d0later,d1) x
    # (r,z,n) x 4 chunks -> column = set*12 + gate*4 + m
    bias_d = nc.dram_tensor("biases", [128, 60], F32, kind="ExternalInput")
    cns_d = nc.dram_tensor("cns", [128, 16], F32, kind="ExternalInput")
    # stacked [W4; W1].T feedback/readout projection, [128, KC, F+1]
    w41t_d = nc.dram_tensor("w41t", [128, KC, F + 1], MMD, kind="ExternalInput")
    out_d = nc.dram_tensor("out", [HORIZONS, BL], F32, kind="ExternalOutput")
    h0o_d = h1o_d = None
    if dump_h:
        h0o_d = nc.dram_tensor("h0o", [128, KC, BL], F32, kind="ExternalOutput")
        h1o_d = nc.dram_tensor("h1o", [128, KC, BL], F32, kind="ExternalOutput")

    with tile.TileContext(nc) as tc:
        with tc.tile_pool(name="wp", bufs=1) as wp, \
             tc.tile_pool(name="sp", bufs=2) as sp, \
             tc.tile_pool(name="hp", bufs=1) as hp, \
             tc.tile_pool(name="gp", bufs=1) as gp, \
             tc.tile_pool(name="op_", bufs=2) as opool, \
             tc.tile_pool(name="pp", bufs=1, space="PSUM") as pp:

            # ---- persistent small tensors (sync queue: ahead of weights) ----
            bias_t = wp.tile([128, 60], F32, tag="bias", name="bias")
            nc.sync.dma_start(bias_t[:], bias_d[:])
            cns_t = wp.tile([128, 16], F32, tag="cns", name="cns")
            nc.sync.dma_start(cns_t[:], cns_d[:])
            ew0a_t = wp.tile([F + 1, G3], MMD, tag="w0a", name="w0a")
            nc.gpsimd.dma_start(ew0a_t[:], ew0a_d[:])

            # first src chunk ahead of any weight bytes on the sync queue
            sc_first = sp.tile([F + 1, SRC_CHUNK, BL], MMD, tag="src", name="src0")
            nc.sync.dma_start(sc_first[:], srcT_d[:, 0:SRC_CHUNK, :])

            # big weight loads round-robin over three DMA queues, in the
            # order the encoder consumes them
            _dmaq = [nc.gpsimd, nc.scalar, nc.sync]
            _dmaq_i = [0]

            def load_u(dram, tagbase):
                ts_ = []
                for k in range(KC):
                    t = wp.tile([128, G3], MMD, tag=f"{tagbase}{k}", name=f"{tagbase}{k}")
                    _dmaq[_dmaq_i[0] % len(_dmaq)].dma_start(t[:], dram[k * 128:(k + 1) * 128, :])
                    _dmaq_i[0] += 1
                    ts_.append(t)
                return ts_

            ew1_t = load_u(w_d["ew1"], "uB")
            eu0_t = load_u(w_d["eu0"], "uA")
            eu1_t = load_u(w_d["eu1"], "uC")
            w41t_t = wp.tile([128, KC, F + 1], MMD, tag="w41t", name="w41t")
            nc.scalar.dma_start(w41t_t[:], w41t_d[:])
            dw0a_t = wp.tile([F + 1, G3], MMD, tag="dw0a", name="dw0a")
            nc.gpsimd.dma_start(dw0a_t[:], dw0a_d[:])
            dw0al_t = wp.tile([F + 1, G3], MMD, tag="dw0al", name="dw0al")
            nc.gpsimd.dma_start(dw0al_t[:], dw0al_d[:])
            dec_w = {}

            # hidden state ping-pong, matmul dtype so matmuls read directly
            h0b = [hp.tile([128, KC, BL], MMD, tag=f"h0{i}", name=f"h0{i}") for i in range(2)]
            h1b = [hp.tile([128, KC, BL], MMD, tag=f"h1{i}", name=f"h1{i}") for i in range(2)]

            def cell(gx_rhs, gx_lhs, gh_lhs, bias_set, cn_col, h_prev, h_out,
                     gh_first=True, skip_gh=False, mm_bias=False):
                """One GRU cell step in gate-major layout.

                gx_rhs: list of rhs APs (K-chunks) for the input projection
                gx_lhs: list of lhsT tiles matching gx_rhs ([*,G3] each)
                gh_lhs: 4 lhsT tiles for the recurrent projection
                bias_set: which column group of bias_t holds this cell's
                          r/z/n biases (applied on the ACT engine, not as
                          rank-1 matmuls)
                cn_col: column in cns_t holding this cell's hidden n-bias (x4)
                gh_first: emit the recurrent (gh) matmuls before the input
                          (gx) ones.  For cells whose gx input is produced by
                          the immediately preceding cell (encoder L1, decoder)
                          this lets the PE chew on gh while the previous
                          cell's activation tail is still in flight.
                """
                pa = [pp.tile([128, 512], F32, tag=f"pA{m}", name=f"pA{m}") for m in range(4)]
                pb = [pp.tile([128, 512], F32, tag=f"pB{m}", name=f"pB{m}") for m in range(4)]
                rz = gp.tile([128, 4, 512], MMD, tag="rz", name="rz")
                tt = gp.tile([128, 4, BL], F32, tag="g1", name="tt")
                vv = gp.tile([128, 4, BL], F32, tag="g2", name="vv")
                nn = gp.tile([128, 4, BL], MMD, tag="gn", name="nn")

                # start=True clears the has_written bits of the WHOLE PSUM
                # bank, so only the chronologically-first matmul into each
                # bank may carry it; later matmuls into untouched columns
                # overwrite cleanly (per-element has_written semantics).
                bank_started = {}

                def _mm(bank_key, out_ap, lhsT, rhs, stop):
                    start = not bank_started.get(bank_key, False)
                    bank_started[bank_key] = True
                    nc.tensor.matmul(out_ap, lhsT, rhs, start=start, stop=stop)

                def gx_part(bank, out_ap, g, stop):
                    for lhs, rhs in zip(gx_lhs, gx_rhs, strict=True):
                        is_last = (lhs is gx_lhs[-1]) and stop
                        _mm(bank, out_ap, lhs[:, g * 128:(g + 1) * 128], rhs, is_last)

                def gh_part(bank, out_ap, g, stop):
                    if skip_gh:
                        return
                    for k in range(len(gh_lhs)):
                        _mm(bank, out_ap, gh_lhs[k][:, g * 128:(g + 1) * 128],
                            h_prev[:, k, :], stop and (k == len(gh_lhs) - 1))

                def bias_ap(gate, m):
                    c = bias_set * 12 + gate * 4 + m
                    return bias_t[:, c:c + 1]

                def sigmoids(m):
                    if mm_bias:
                        # biases already accumulated via the gx ones-row:
                        # one wide sigmoid covers r and z
                        nc.scalar.activation(rz[:, m, :], pa[m][:, :], AF.Sigmoid)
                    else:
                        nc.scalar.activation(rz[:, m, 0:BL], pa[m][:, 0:BL],
                                             AF.Sigmoid, bias=bias_ap(0, m))
                        nc.scalar.activation(rz[:, m, BL:2 * BL], pa[m][:, BL:2 * BL],
                                             AF.Sigmoid, bias=bias_ap(1, m))

                if gh_first:
                    # phase 1: everything that only needs h_prev.  pb banks
                    # last: the previous cell's stt/tt reads of them finish
                    # latest, so this ordering avoids bank-WAR stalls.
                    for m in range(4):
                        gh_part(("a", m), pa[m][:, 0:BL], m, False)          # r
                        gh_part(("a", m), pa[m][:, BL:2 * BL], 4 + m, False)  # z
                    for m in range(4):
                        gh_part(("b", m), pb[m][:, BL:2 * BL], 8 + m, True)   # hn
                    # phase 2: the gx half (input produced by previous cell)
                    for m in range(4):
                        gx_part(("a", m), pa[m][:, 0:BL], m, True)
                        gx_part(("a", m), pa[m][:, BL:2 * BL], 4 + m, True)
                        sigmoids(m)
                        gx_part(("b", m), pb[m][:, 0:BL], 8 + m, True)        # xn
                        if skip_gh:
                            nc.vector.memset(pb[m][:, BL:2 * BL], 0.0)
                else:
                    for m in range(4):
                        gx_part(("a", m), pa[m][:, 0:BL], m, skip_gh)
                        gx_part(("a", m), pa[m][:, BL:2 * BL], 4 + m, skip_gh)
                    for m in range(4):
                        gx_part(("b", m), pb[m][:, 0:BL], 8 + m, True)
                    for m in range(4):
                        gh_part(("a", m), pa[m][:, 0:BL], m, True)
                        gh_part(("a", m), pa[m][:, BL:2 * BL], 4 + m, True)
                        sigmoids(m)
                        gh_part(("b", m), pb[m][:, BL:2 * BL], 8 + m, True)   # hn
                        if skip_gh:
                            nc.vector.memset(pb[m][:, BL:2 * BL], 0.0)

                # activation tail, pipelined per m-chunk
                for m in range(4):
                    # t = (hn + cn) * r
                    nc.vector.scalar_tensor_tensor(
                        tt[:, m, :], pb[m][:, BL:2 * BL], cns_t[:, cn_col * 4 + m:cn_col * 4 + m + 1],
                        rz[:, m, 0:BL], OP.add, OP.mult)
                    # v = t + xn
                    nc.vector.tensor_tensor(vv[:, m, :], tt[:, m, :], pb[m][:, 0:BL], OP.add)
                    nc.scalar.activation(nn[:, m, :], vv[:, m, :], AF.Tanh,
                                         bias=0.0 if mm_bias else bias_ap(2, m))
                # h' = n + z*(h - n), per-chunk so consumers of the early
                # chunks unblock as soon as possible
                dd = gp.tile([128, 4, BL], MMD, tag="g1", name="dd")
                ee = gp.tile([128, 4, BL], MMD, tag="g2", name="ee")
                for m in range(4):
                    sl = slice(m, m + 1)
                    nc.vector.tensor_tensor(dd[:, sl, :], h_prev[:, sl, :], nn[:, sl, :], OP.subtract)
                    nc.vector.tensor_tensor(ee[:, sl, :], rz[:, sl, BL:2 * BL], dd[:, sl, :], OP.mult)
                    nc.vector.tensor_tensor(h_out[:, sl, :], ee[:, sl, :], nn[:, sl, :], OP.add)

            for _rep in range(repeat):
                for i in range(2):
                    nc.vector.memzero(h0b[i][:])
                    nc.vector.memzero(h1b[i][:])

                # ---------------- encoder ----------------
                sc = None
                for t in range(lags):
                    if t == 0 and _rep == 0:
                        sc = sc_first
                    elif t % SRC_CHUNK == 0:
                        sc = sp.tile([F + 1, SRC_CHUNK, BL], MMD, tag="src", name=f"src{t}")
                        nc.sync.dma_start(
                            sc[:], srcT_d[:, t:t + SRC_CHUNK, :])
                    j = t % SRC_CHUNK
                    p, q = t % 2, (t + 1) % 2
                    cell([sc[:, j, :]], [ew0a_t], eu0_t, 0, 0, h0b[p], h0b[q],
                         gh_first=False, skip_gh=(t == 0), mm_bias=True)
                    cell([h0b[q][:, k, :] for k in range(KC)], ew1_t, eu1_t, 1, 1,
                         h1b[p], h1b[q], skip_gh=(t == 0))
                    if t == 1 and not dec_w:
                        # decoder weights DMA behind the encoder-critical
                        # loads; they have ~1.2ms to arrive
                        dec_w["du0"] = load_u(w_d["du0"], "uE")
                        dec_w["dw1"] = load_u(w_d["dw1"], "uF")
                        dec_w["du1"] = load_u(w_d["du1"], "uG")
                sc_last = sc
                du0_t, dw1_t, du1_t = dec_w["du0"], dec_w["dw1"], dec_w["du1"]
                if dump_h:
                    pfin = lags % 2
                    h0c = gp.tile([128, KC, BL], F32, tag="g1", name="h0c")
                    nc.vector.tensor_copy(h0c[:], h0b[pfin][:, 0:KC, :])
                    nc.sync.dma_start(h0o_d[:], h0c[:])
                    h1c = gp.tile([128, KC, BL], F32, tag="g2", name="h1c")
                    nc.vector.tensor_copy(h1c[:], h1b[pfin][:, 0:KC, :])
                    nc.sync.dma_start(h1o_d[:], h1c[:])

                # ---------------- decoder ----------------
                # dW0@W4 is rank-64, so the fc4 feedback runs two-stage:
                # u = [W4; W1] @ h1 (4 matmuls; row 64 doubles as out1),
                # then gx = dW0 @ u (12 K=64 matmuls) inside the next cell.
                u_sb = None
                for d in range(horizons):
                    p, q = (lags + d) % 2, (lags + d + 1) % 2
                    if d == 0:
                        cell([sc_last[:, (lags - 1) % SRC_CHUNK, :]], [dw0a_t],
                             du0_t, 2, 2, h0b[p], h0b[q], gh_first=False,
                             mm_bias=True)
                    else:
                        cell([u_sb[:]], [dw0al_t], du0_t, 3, 2, h0b[p], h0b[q],
                             mm_bias=True)
                    cell([h0b[q][:, k, :] for k in range(KC)], dw1_t, du1_t, 4, 3,
                         h1b[p], h1b[q])
                    # uo = [W4; W1] . h1_new  (b1 added on host; b4 is folded
                    # into the d0later bias set via dW0@b4)
                    po = pp.tile([128, 512], F32, tag="pB3", name=f"po{d}")
                    for k in range(KC):
                        nc.tensor.matmul(po[0:F + 1, 0:BL], w41t_t[:, k, :],
                                         h1b[q][:, k, :], start=(k == 0), stop=(k == KC - 1))
                    if d + 1 < horizons:
                        # u feeds the next cell's gx: copy it first, on ACT.
                        # row F is a ones-row that applies the d0later biases
                        u_sb = opool.tile([F + 1, BL], MMD, tag="usb", name=f"u{d}")
                        nc.scalar.copy(u_sb[0:F, :], po[0:F, 0:BL])
                        nc.vector.memset(u_sb[F:F + 1, :], 1.0)
                    osb = opool.tile([1, BL], F32, tag="o1", name=f"o{d}")
                    nc.vector.tensor_copy(osb[:], po[F:F + 1, 0:BL])
                    nc.sync.dma_start(out_d[d:d + 1, :], osb[:])

    nc.compile()
    return nc


def _host_prep(inputs):
    import ml_dtypes
    f32 = np.float32
    bf16 = ml_dtypes.bfloat16
    g = {k: np.asarray(v, dtype=f32) if np.asarray(v).dtype != np.int64 else v
         for k, v in inputs.items()}
    src = np.asarray(inputs["src"], f32)
    eW0, eU0, eb0, ec0 = g["eW0"], g["eU0"], g["eb0"], g["ec0"]
    eW1, eU1, eb1, ec1 = g["eW1"], g["eU1"], g["eb1"], g["ec1"]
    dW0, dU0, db0, dc0 = g["dW0"], g["dU0"], g["db0"], g["dc0"]
    dW1, dU1, db1, dc1 = g["dW1"], g["dU1"], g["db1"], g["dc1"]
    W1, b1, W4, b4 = g["W1"], g["b1"], g["W4"], g["b4"]

    def rzn_bias(b, c):
        return np.concatenate([b[0:H] + c[0:H], b[H:2 * H] + c[H:2 * H], b[2 * H:]])

    dcomb = (db0 + dW0 @ b4).astype(f32)                 # [1536]
    W41T = np.concatenate([W4, W1[0:1]], 0).T.copy()     # [512, 65]
    shared = {
        "eu0": eU0.T.copy(), "ew1": eW1.T.copy(), "eu1": eU1.T.copy(),
        "du0": dU0.T.copy(), "dw1": dW1.T.copy(), "du1": dU1.T.copy(),
        "ew0a": np.concatenate([eW0.T, rzn_bias(eb0, ec0)[None, :]], 0),
        "dw0a": np.concatenate([dW0.T, rzn_bias(db0, dc0)[None, :]], 0),
        "dw0al": np.concatenate([dW0.T, rzn_bias(dcomb, dc0)[None, :]], 0),
        "w41t": W41T.reshape(KC, 128, F + 1).transpose(1, 0, 2),
    }
    # cns layout: cns[:, c*4+m] = c_n[m*128+p]
    cns = np.zeros((128, 16), f32)
    for ci, c in enumerate((ec0, ec1, dc0, dc1)):
        cn = c[2 * H:]
        for m in range(KC):
            cns[:, ci * 4 + m] = cn[m * 128:(m + 1) * 128]
    shared["cns"] = cns
    # ACT-applied biases: 5 sets x (r,z,n) x 4 chunks; col = set*12+gate*4+m
    biases = np.zeros((128, 60), f32)
    sets = [rzn_bias(eb0, ec0), rzn_bias(eb1, ec1), rzn_bias(db0, dc0),
            rzn_bias(dcomb, dc0), rzn_bias(db1, dc1)]
    for si, full in enumerate(sets):
        for g in range(3):
            for m in range(KC):
                biases[:, si * 12 + g * 4 + m] = full[g * H + m * 128:(g * H + (m + 1) * 128)]
    shared["biases"] = biases
    _f32_keys = ("cns", "biases")
    shared = {k: np.ascontiguousarray(v, dtype=(f32 if k in _f32_keys else bf16))
              for k, v in shared.items()}

    in_maps = []
    for c in range(NCORES):
        s = src[c * BL:(c + 1) * BL]                     # [256, 64, 64]
        sT = s.transpose(2, 1, 0)           # [64, 64, 256]
        sA = np.concatenate([sT, np.ones((1, LAGS, BL), np.float32)], 0)
        m = dict(shared)
        m["srcT"] = np.ascontiguousarray(sA.astype(bf16))
        in_maps.append(m)
    return in_maps, float(b1[0])


class _Runner:
    """Build-once sharded PJRT runner (axon: 8 NeuronCores)."""

    def __init__(self, nc):
        import jax
        from jax.sharding import Mesh, PartitionSpec
        from jax.experimental.shard_map import shard_map
        from concourse import mybir
        from concourse.bass2jax import (_bass_exec_p, partition_id_tensor,
                                        install_neuronx_cc_hook)
        install_neuronx_cc_hook()
        self.jax = jax
        partition_name = nc.partition_id_tensor.name if nc.partition_id_tensor else None
        in_names, out_names, out_avals, zero_outs = [], [], [], []
        for alloc in nc.m.functions[0].allocations:
            if not isinstance(alloc, mybir.MemoryLocationSet):
                continue
            name = alloc.memorylocations[0].name
            if alloc.kind == "ExternalInput":
                if name != partition_name:
                    in_names.append(name)
            elif alloc.kind == "ExternalOutput":
                out_names.append(name)
                shape = tuple(alloc.tensor_shape)
                dtype = mybir.dt.np(alloc.dtype)
                out_avals.append(jax.core.ShapedArray(shape, dtype))
                zero_outs.append(np.zeros(shape, dtype))
        n_params = len(in_names)
        all_in = list(in_names) + list(out_names)
        if partition_name is not None:
            all_in.append(partition_name)
        self.in_names, self.out_names = in_names, out_names
        self.out_avals, self.zero_outs = out_avals, zero_outs

        def _body(*args):
            operands = list(args)
            if partition_name is not None:
                operands.append(partition_id_tensor())
            return tuple(_bass_exec_p.bind(
                *operands, out_avals=tuple(out_avals), in_names=tuple(all_in),
                out_names=tuple(out_names), lowering_input_output_aliases=(),
                sim_require_finite=True, sim_require_nnan=True, nc=nc))

        devices = jax.devices()[:NCORES]
        self.mesh = Mesh(np.asarray(devices), ("core",))
        in_specs = (PartitionSpec("core"),) * (n_params + len(out_names))
        out_specs = (PartitionSpec("core"),) * len(out_names)
        donate = tuple(range(n_params, n_params + len(out_names)))
        self.fn = jax.jit(
            shard_map(_body, mesh=self.mesh, in_specs=in_specs,
                      out_specs=out_specs, check_rep=False),
            donate_argnums=donate, keep_unused=True)
        self.sh = jax.sharding.NamedSharding(self.mesh, PartitionSpec("core"))

    def place(self, in_maps):
        n = NCORES
        self.placed = [
            self.jax.device_put(np.ascontiguousarray(
                np.concatenate([in_maps[c][nm] for c in range(n)], 0)), self.sh)
            for nm in self.in_names]

    def run(self):
        zeros = [self.jax.device_put(
            np.zeros((NCORES * z.shape[0], *z.shape[1:]), z.dtype), self.sh)
            for z in self.zero_outs]
        outs = self.fn(*self.placed, *zeros)
        self.jax.block_until_ready(outs)
        return outs

    def results(self, outs):
        return [
            {nm: np.asarray(outs[i]).reshape(NCORES, *self.out_avals[i].shape)[c]
             for i, nm in enumerate(self.out_names)}
            for c in range(NCORES)]


def get_runner(repeat=1):
    global _RUNNER
    key = ("r", repeat)
    if _RUNNER is None or _RUNNER[0] != key:
        nc = _build_nc(repeat=repeat)
        _RUNNER = (key, _Runner(nc))
    return _RUNNER[1]


def kernel(**inputs) -> np.ndarray:
    global _RUNNER
    in_maps, b1 = _host_prep(inputs)
    res = None
    for attempt in range(3):
        try:
            r = get_runner()
            r.place(in_maps)
            res = r.results(r.run())
            break
        except Exception:
            # transient NRT device errors have been observed; rebuild once
            _RUNNER = None
            if attempt == 2:
                raise
            import time
            time.sleep(10 + 20 * attempt)
    out = np.empty((B, HORIZONS), np.float32)
    for c in range(NCORES):
        out[c * BL:(c + 1) * BL] = res[c]["out"].T + b1
    return out



# revision 30
# speedup vs baseline: 1.0053x; 1.0053x over previous
"""GRU Seq2Seq Trainium2 kernel (nn_GRU_Seq2Seq_83219286327778).

Strategy: data-parallel over batch (2048 -> 8 x 256), gate-major transposed
layout on-device ([hidden/gate dim on partitions, batch on free dim]) so the
recurrence needs no transposes.  gx+gh accumulate in PSUM; gate biases are
applied via ACT-engine bias operands (no rank-1 bias matmuls).  Per cell the
matmuls whose operand is already available (recurrent gh for L1/decoder
cells) are emitted first so the PE stays busy across every activation tail
(keeps HAM at full clock).  The decoder fc4 feedback dW0@W4 is rank-64 and
runs two-stage: u = [W4;W1]@h1 (the W1 row doubles as the out1 readout),
then gx = dW0@u.  Matmuls + elementwise run in bf16 (fp32 PSUM accumulate).
"""
import sys
sys.path.insert(0, "/opt/trn_rl_repo")
import numpy as np

B, LAGS, HORIZONS, F, H = 2048, 64, 24, 64, 512
NCORES = 8
BL = B // NCORES           # 256 batch per core
G3 = 3 * H                 # 1536
KC = H // 128              # 4 k-chunks
SRC_CHUNK = 8              # timesteps per src DMA

_RUNNER = None


def _build_nc(mm_dt_name="bfloat16", repeat=1, lags=LAGS, horizons=HORIZONS, dump_h=False):
    import concourse.tile as tile
    from concourse import mybir, bacc

    F32 = mybir.dt.float32
    MMD = getattr(mybir.dt, mm_dt_name)
    AF = mybir.ActivationFunctionType
    OP = mybir.AluOpType

    nc = bacc.Bacc("TRN2", target_bir_lowering=False)

    srcT_d = nc.dram_tensor("srcT", [F + 1, LAGS, BL], MMD, kind="ExternalInput")
    wnames = ["eu0", "ew1", "eu1", "du0", "dw1", "du1"]
    w_d = {n: nc.dram_tensor(n, [H, G3], MMD, kind="ExternalInput") for n in wnames}
    ew0a_d = nc.dram_tensor("ew0a", [F + 1, G3], MMD, kind="ExternalInput")
    dw0a_d = nc.dram_tensor("dw0a", [F + 1, G3], MMD, kind="ExternalInput")
    dw0al_d = nc.dram_tensor("dw0al", [F + 1, G3], MMD, kind="ExternalInput")
    # per-partition activation biases: 5 sets (e0,e1,d0first,d0later,d1) x
    # (r,z,n) x 4 chunks -> column = set*12 + gate*4 + m
    bias_d = nc.dram_tensor("biases", [128, 60], F32, kind="ExternalInput")
    cns_d = nc.dram_tensor("cns", [128, 16], F32, kind="ExternalInput")
    # stacked [W4; W1].T feedback/readout projection, [128, KC, F+1]
    w41t_d = nc.dram_tensor("w41t", [128, KC, F + 1], MMD, kind="ExternalInput")
    out_d = nc.dram_tensor("out", [HORIZONS, BL], F32, kind="ExternalOutput")
    h0o_d = h1o_d = None
    if dump_h:
        h0o_d = nc.dram_tensor("h0o", [128, KC, BL], F32, kind="ExternalOutput")
        h1o_d = nc.dram_tensor("h1o", [128, KC, BL], F32, kind="ExternalOutput")

    with tile.TileContext(nc) as tc:
        with tc.tile_pool(name="wp", bufs=1) as wp, \
             tc.tile_pool(name="sp", bufs=2) as sp, \
             tc.tile_pool(name="hp", bufs=1) as hp, \
             tc.tile_pool(name="gp", bufs=1) as gp, \
             tc.tile_pool(name="op_", bufs=2) as opool, \
             tc.tile_pool(name="pp", bufs=1, space="PSUM") as pp:

            # ---- persistent small tensors (sync queue: ahead of weights) ----
            bias_t = wp.tile([128, 60], F32, tag="bias", name="bias")
            nc.sync.dma_start(bias_t[:], bias_d[:])
            cns_t = wp.tile([128, 16], F32, tag="cns", name="cns")
            nc.sync.dma_start(cns_t[:], cns_d[:])
            ew0a_t = wp.tile([F + 1, G3], MMD, tag="w0a", name="w0a")
            nc.gpsimd.dma_start(ew0a_t[:], ew0a_d[:])

            # big weight loads round-robin on gpsimd/scalar, in the order the
            # encoder consumes them; src + small stay on sync
            _dmaq = [nc.gpsimd, nc.scalar]
            _dmaq_i = [0]

            def load_u(dram, tagbase):
                ts_ = []
                for k in range(KC):
                    t = wp.tile([128, G3], MMD, tag=f"{tagbase}{k}", name=f"{tagbase}{k}")
                    _dmaq[_dmaq_i[0] % len(_dmaq)].dma_start(t[:], dram[k * 128:(k + 1) * 128, :])
                    _dmaq_i[0] += 1
                    ts_.append(t)
                return ts_

            ew1_t = load_u(w_d["ew1"], "uB")
            eu0_t = load_u(w_d["eu0"], "uA")
            eu1_t = load_u(w_d["eu1"], "uC")
            w41t_t = wp.tile([128, KC, F + 1], MMD, tag="w41t", name="w41t")
            nc.scalar.dma_start(w41t_t[:], w41t_d[:])
            dw0a_t = wp.tile([F + 1, G3], MMD, tag="dw0a", name="dw0a")
            nc.gpsimd.dma_start(dw0a_t[:], dw0a_d[:])
            dw0al_t = wp.tile([F + 1, G3], MMD, tag="dw0al", name="dw0al")
            nc.gpsimd.dma_start(dw0al_t[:], dw0al_d[:])
            dec_w = {}

            # hidden state ping-pong, matmul dtype so matmuls read directly
            h0b = [hp.tile([128, KC, BL], MMD, tag=f"h0{i}", name=f"h0{i}") for i in range(2)]
            h1b = [hp.tile([128, KC, BL], MMD, tag=f"h1{i}", name=f"h1{i}") for i in range(2)]

            def cell(gx_rhs, gx_lhs, gh_lhs, bias_set, cn_col, h_prev, h_out,
                     gh_first=True, skip_gh=False, mm_bias=False):
                """One GRU cell step in gate-major layout.

                gx_rhs: list of rhs APs (K-chunks) for the input projection
                gx_lhs: list of lhsT tiles matching gx_rhs ([*,G3] each)
                gh_lhs: 4 lhsT tiles for the recurrent projection
                bias_set: which column group of bias_t holds this cell's
                          r/z/n biases (applied on the ACT engine, not as
                          rank-1 matmuls)
                cn_col: column in cns_t holding this cell's hidden n-bias (x4)
                gh_first: emit the recurrent (gh) matmuls before the input
                          (gx) ones.  For cells whose gx input is produced by
                          the immediately preceding cell (encoder L1, decoder)
                          this lets the PE chew on gh while the previous
                          cell's activation tail is still in flight.
                """
                pa = [pp.tile([128, 512], F32, tag=f"pA{m}", name=f"pA{m}") for m in range(4)]
                pb = [pp.tile([128, 512], F32, tag=f"pB{m}", name=f"pB{m}") for m in range(4)]
                rz = gp.tile([128, 4, 512], MMD, tag="rz", name="rz")
                tt = gp.tile([128, 4, BL], F32, tag="g1", name="tt")
                vv = gp.tile([128, 4, BL], F32, tag="g2", name="vv")
                nn = gp.tile([128, 4, BL], MMD, tag="gn", name="nn")

                # start=True clears the has_written bits of the WHOLE PSUM
                # bank, so only the chronologically-first matmul into each
                # bank may carry it; later matmuls into untouched columns
                # overwrite cleanly (per-element has_written semantics).
                bank_started = {}

                def _mm(bank_key, out_ap, lhsT, rhs, stop):
                    start = not bank_started.get(bank_key, False)
                    bank_started[bank_key] = True
                    nc.tensor.matmul(out_ap, lhsT, rhs, start=start, stop=stop)

                def gx_part(bank, out_ap, g, stop):
                    for lhs, rhs in zip(gx_lhs, gx_rhs, strict=True):
                        is_last = (lhs is gx_lhs[-1]) and stop
                        _mm(bank, out_ap, lhs[:, g * 128:(g + 1) * 128], rhs, is_last)

                def gh_part(bank, out_ap, g, stop):
                    if skip_gh:
                        return
                    for k in range(len(gh_lhs)):
                        _mm(bank, out_ap, gh_lhs[k][:, g * 128:(g + 1) * 128],
                            h_prev[:, k, :], stop and (k == len(gh_lhs) - 1))

                def bias_ap(gate, m):
                    c = bias_set * 12 + gate * 4 + m
                    return bias_t[:, c:c + 1]

                def sigmoids(m):
                    if mm_bias:
                        # biases already accumulated via the gx ones-row:
                        # one wide sigmoid covers r and z
                        nc.scalar.activation(rz[:, m, :], pa[m][:, :], AF.Sigmoid)
                    else:
                        nc.scalar.activation(rz[:, m, 0:BL], pa[m][:, 0:BL],
                                             AF.Sigmoid, bias=bias_ap(0, m))
                        nc.scalar.activation(rz[:, m, BL:2 * BL], pa[m][:, BL:2 * BL],
                                             AF.Sigmoid, bias=bias_ap(1, m))

                if gh_first:
                    # phase 1: everything that only needs h_prev.  pb banks
                    # last: the previous cell's stt/tt reads of them finish
                    # latest, so this ordering avoids bank-WAR stalls.
                    for m in range(4):
                        gh_part(("a", m), pa[m][:, 0:BL], m, False)          # r
                        gh_part(("a", m), pa[m][:, BL:2 * BL], 4 + m, False)  # z
                    for m in range(4):
                        gh_part(("b", m), pb[m][:, BL:2 * BL], 8 + m, True)   # hn
                    # phase 2: the gx half (input produced by previous cell)
                    for m in range(4):
                        gx_part(("a", m), pa[m][:, 0:BL], m, True)
                        gx_part(("a", m), pa[m][:, BL:2 * BL], 4 + m, True)
                        sigmoids(m)
                        gx_part(("b", m), pb[m][:, 0:BL], 8 + m, True)        # xn
                        if skip_gh:
                            nc.vector.memset(pb[m][:, BL:2 * BL], 0.0)
                else:
                    for m in range(4):
                        gx_part(("a", m), pa[m][:, 0:BL], m, skip_gh)
                        gx_part(("a", m), pa[m][:, BL:2 * BL], 4 + m, skip_gh)
                    for m in range(4):
                        gx_part(("b", m), pb[m][:, 0:BL], 8 + m, True)
                    for m in range(4):
                        gh_part(("a", m), pa[m][:, 0:BL], m, True)
                        gh_part(("a", m), pa[m][:, BL:2 * BL], 4 + m, True)
                        sigmoids(m)
                        gh_part(("b", m), pb[m][:, BL:2 * BL], 8 + m, True)   # hn
                        if skip_gh:
                            nc.vector.memset(pb[m][:, BL:2 * BL], 0.0)

                # activation tail, pipelined per m-chunk
                for m in range(4):
                    # t = (hn + cn) * r
                    nc.vector.scalar_tensor_tensor(
                        tt[:, m, :], pb[m][:, BL:2 * BL], cns_t[:, cn_col * 4 + m:cn_col * 4 + m + 1],
                        rz[:, m, 0:BL], OP.add, OP.mult)
                    # v = t + xn
                    nc.vector.tensor_tensor(vv[:, m, :], tt[:, m, :], pb[m][:, 0:BL], OP.add)
                    nc.scalar.activation(nn[:, m, :], vv[:, m, :], AF.Tanh,
                                         bias=0.0 if mm_bias else bias_ap(2, m))
                # h' = n + z*(h - n), per-chunk so consumers of the early
                # chunks unblock as soon as possible
                dd = gp.tile([128, 4, BL], MMD, tag="g1", name="dd")
                ee = gp.tile([128, 4, BL], MMD, tag="g2", name="ee")
                for m in range(4):
                    sl = slice(m, m + 1)
                    nc.vector.tensor_tensor(dd[:, sl, :], h_prev[:, sl, :], nn[:, sl, :], OP.subtract)
                    nc.vector.tensor_tensor(ee[:, sl, :], rz[:, sl, BL:2 * BL], dd[:, sl, :], OP.mult)
                    nc.vector.tensor_tensor(h_out[:, sl, :], ee[:, sl, :], nn[:, sl, :], OP.add)

            for _rep in range(repeat):
                for i in range(2):
                    nc.vector.memzero(h0b[i][:])
                    nc.vector.memzero(h1b[i][:])

                # ---------------- encoder ----------------
                sc = None
                for t in range(lags):
                    if t % SRC_CHUNK == 0:
                        sc = sp.tile([F + 1, SRC_CHUNK, BL], MMD, tag="src", name=f"src{t}")
                        nc.sync.dma_start(
                            sc[:], srcT_d[:, t:t + SRC_CHUNK, :])
                    j = t % SRC_CHUNK
                    p, q = t % 2, (t + 1) % 2
                    cell([sc[:, j, :]], [ew0a_t], eu0_t, 0, 0, h0b[p], h0b[q],
                         gh_first=False, skip_gh=(t == 0), mm_bias=True)
                    cell([h0b[q][:, k, :] for k in range(KC)], ew1_t, eu1_t, 1, 1,
                         h1b[p], h1b[q], skip_gh=(t == 0))
                    if t == 1 and not dec_w:
                        # decoder weights DMA behind the encoder-critical
                        # loads; they have ~1.2ms to arrive
                        dec_w["du0"] = load_u(w_d["du0"], "uE")
                        dec_w["dw1"] = load_u(w_d["dw1"], "uF")
                        dec_w["du1"] = load_u(w_d["du1"], "uG")
                sc_last = sc
                du0_t, dw1_t, du1_t = dec_w["du0"], dec_w["dw1"], dec_w["du1"]
                if dump_h:
                    pfin = lags % 2
                    h0c = gp.tile([128, KC, BL], F32, tag="g1", name="h0c")
                    nc.vector.tensor_copy(h0c[:], h0b[pfin][:, 0:KC, :])
                    nc.sync.dma_start(h0o_d[:], h0c[:])
                    h1c = gp.tile([128, KC, BL], F32, tag="g2", name="h1c")
                    nc.vector.tensor_copy(h1c[:], h1b[pfin][:, 0:KC, :])
                    nc.sync.dma_start(h1o_d[:], h1c[:])

                # ---------------- decoder ----------------
                # dW0@W4 is rank-64, so the fc4 feedback runs two-stage:
                # u = [W4; W1] @ h1 (4 matmuls; row 64 doubles as out1),
                # then gx = dW0 @ u (12 K=64 matmuls) inside the next cell.
                u_sb = None
                for d in range(horizons):
                    p, q = (lags + d) % 2, (lags + d + 1) % 2
                    if d == 0:
                        cell([sc_last[:, (lags - 1) % SRC_CHUNK, :]], [dw0a_t],
                             du0_t, 2, 2, h0b[p], h0b[q], gh_first=False,
                             mm_bias=True)
                    else:
                        cell([u_sb[:]], [dw0al_t], du0_t, 3, 2, h0b[p], h0b[q],
                             mm_bias=True)
                    cell([h0b[q][:, k, :] for k in range(KC)], dw1_t, du1_t, 4, 3,
                         h1b[p], h1b[q])
                    # uo = [W4; W1] . h1_new  (b1 added on host; b4 is folded
                    # into the d0later bias set via dW0@b4)
                    po = pp.tile([128, 512], F32, tag="pA0", name=f"po{d}")
                    for k in range(KC):
                        nc.tensor.matmul(po[0:F + 1, 0:BL], w41t_t[:, k, :],
                                         h1b[q][:, k, :], start=(k == 0), stop=(k == KC - 1))
                    if d + 1 < horizons:
                        # u feeds the next cell's gx: copy it first, on ACT.
                        # row F is a ones-row that applies the d0later biases
                        u_sb = opool.tile([F + 1, BL], MMD, tag="usb", name=f"u{d}")
                        nc.scalar.copy(u_sb[0:F, :], po[0:F, 0:BL])
                        nc.vector.memset(u_sb[F:F + 1, :], 1.0)
                    osb = opool.tile([1, BL], F32, tag="o1", name=f"o{d}")
                    nc.vector.tensor_copy(osb[:], po[F:F + 1, 0:BL])
                    nc.sync.dma_start(out_d[d:d + 1, :], osb[:])

    nc.compile()
    return nc


def _host_prep(inputs):
    import ml_dtypes
    f32 = np.float32
    bf16 = ml_dtypes.bfloat16
    g = {k: np.asarray(v, dtype=f32) if np.asarray(v).dtype != np.int64 else v
         for k, v in inputs.items()}
    src = np.asarray(inputs["src"], f32)
    eW0, eU0, eb0, ec0 = g["eW0"], g["eU0"], g["eb0"], g["ec0"]
    eW1, eU1, eb1, ec1 = g["eW1"], g["eU1"], g["eb1"], g["ec1"]
    dW0, dU0, db0, dc0 = g["dW0"], g["dU0"], g["db0"], g["dc0"]
    dW1, dU1, db1, dc1 = g["dW1"], g["dU1"], g["db1"], g["dc1"]
    W1, b1, W4, b4 = g["W1"], g["b1"], g["W4"], g["b4"]

    def rzn_bias(b, c):
        return np.concatenate([b[0:H] + c[0:H], b[H:2 * H] + c[H:2 * H], b[2 * H:]])

    dcomb = (db0 + dW0 @ b4).astype(f32)                 # [1536]
    W41T = np.concatenate([W4, W1[0:1]], 0).T.copy()     # [512, 65]
    shared = {
        "eu0": eU0.T.copy(), "ew1": eW1.T.copy(), "eu1": eU1.T.copy(),
        "du0": dU0.T.copy(), "dw1": dW1.T.copy(), "du1": dU1.T.copy(),
        "ew0a": np.concatenate([eW0.T, rzn_bias(eb0, ec0)[None, :]], 0),
        "dw0a": np.concatenate([dW0.T, rzn_bias(db0, dc0)[None, :]], 0),
        "dw0al": np.concatenate([dW0.T, rzn_bias(dcomb, dc0)[None, :]], 0),
        "w41t": W41T.reshape(KC, 128, F + 1).transpose(1, 0, 2),
    }
    # cns layout: cns[:, c*4+m] = c_n[m*128+p]
    cns = np.zeros((128, 16), f32)
    for ci, c in enumerate((ec0, ec1, dc0, dc1)):
        cn = c[2 * H:]
        for m in range(KC):
            cns[:, ci * 4 + m] = cn[m * 128:(m + 1) * 128]
    shared["cns"] = cns
    # ACT-applied biases: 5 sets x (r,z,n) x 4 chunks; col = set*12+gate*4+m
    biases = np.zeros((128, 60), f32)
    sets = [rzn_bias(eb0, ec0), rzn_bias(eb1, ec1), rzn_bias(db0, dc0),
            rzn_bias(dcomb, dc0), rzn_bias(db1, dc1)]
    for si, full in enumerate(sets):
        for g in range(3):
            for m in range(KC):
                biases[:, si * 12 + g * 4 + m] = full[g * H + m * 128:(g * H + (m + 1) * 128)]
    shared["biases"] = biases
    _f32_keys = ("cns", "biases")
    shared = {k: np.ascontiguousarray(v, dtype=(f32 if k in _f32_keys else bf16))
              for k, v in shared.items()}

    in_maps = []
    for c in range(NCORES):
        s = src[c * BL:(c + 1) * BL]                     # [256, 64, 64]
        sT = s.transpose(2, 1, 0)           # [64, 64, 256]
        sA = np.concatenate([sT, np.ones((1, LAGS, BL), np.float32)], 0)
        m = dict(shared)
        m["srcT"] = np.ascontiguousarray(sA.astype(bf16))
        in_maps.append(m)
    return in_maps, float(b1[0])


class _Runner:
    """Build-once sharded PJRT runner (axon: 8 NeuronCores)."""

    def __init__(self, nc):
        import jax
        from jax.sharding import Mesh, PartitionSpec
        from jax.experimental.shard_map import shard_map
        from concourse import mybir
        from concourse.bass2jax import (_bass_exec_p, partition_id_tensor,
                                        install_neuronx_cc_hook)
        install_neuronx_cc_hook()
        self.jax = jax
        partition_name = nc.partition_id_tensor.name if nc.partition_id_tensor else None
        in_names, out_names, out_avals, zero_outs = [], [], [], []
        for alloc in nc.m.functions[0].allocations:
            if not isinstance(alloc, mybir.MemoryLocationSet):
                continue
            name = alloc.memorylocations[0].name
            if alloc.kind == "ExternalInput":
                if name != partition_name:
                    in_names.append(name)
            elif alloc.kind == "ExternalOutput":
                out_names.append(name)
                shape = tuple(alloc.tensor_shape)
                dtype = mybir.dt.np(alloc.dtype)
                out_avals.append(jax.core.ShapedArray(shape, dtype))
                zero_outs.append(np.zeros(shape, dtype))
        n_params = len(in_names)
        all_in = list(in_names) + list(out_names)
        if partition_name is not None:
            all_in.append(partition_name)
        self.in_names, self.out_names = in_names, out_names
        self.out_avals, self.zero_outs = out_avals, zero_outs

        def _body(*args):
            operands = list(args)
            if partition_name is not None:
                operands.append(partition_id_tensor())
            return tuple(_bass_exec_p.bind(
                *operands, out_avals=tuple(out_avals), in_names=tuple(all_in),
                out_names=tuple(out_names), lowering_input_output_aliases=(),
                sim_require_finite=True, sim_require_nnan=True, nc=nc))

        devices = jax.devices()[:NCORES]
        self.mesh = Mesh(np.asarray(devices), ("core",))
        in_specs = (PartitionSpec("core"),) * (n_params + len(out_names))
        out_specs = (PartitionSpec("core"),) * len(out_names)
        donate = tuple(range(n_params, n_params + len(out_names)))
        self.fn = jax.jit(
            shard_map(_body, mesh=self.mesh, in_specs=in_specs,
                      out_specs=out_specs, check_rep=False),
            donate_argnums=donate, keep_unused=True)
        self.sh = jax.sharding.NamedSharding(self.mesh, PartitionSpec("core"))

    def place(self, in_maps):
        n = NCORES
        self.placed = [
            self.jax.device_put(np.ascontiguousarray(
                np.concatenate([in_maps[c][nm] for c in range(n)], 0)), self.sh)
            for nm in self.in_names]

    def run(self):
        zeros = [self.jax.device_put(
            np.zeros((NCORES * z.shape[0], *z.shape[1:]), z.dtype), self.sh)
            for z in self.zero_outs]
        outs = self.fn(*self.placed, *zeros)
        self.jax.block_until_ready(outs)
        return outs

    def results(self, outs):
        return [
            {nm: np.asarray(outs[i]).reshape(NCORES, *self.out_avals[i].shape)[c]
             for i, nm in enumerate(self.out_names)}
            for c in range(NCORES)]


def get_runner(repeat=1):
    global _RUNNER
    key = ("r", repeat)
    if _RUNNER is None or _RUNNER[0] != key:
        nc = _build_nc(repeat=repeat)
        _RUNNER = (key, _Runner(nc))
    return _RUNNER[1]


def kernel(**inputs) -> np.ndarray:
    global _RUNNER
    in_maps, b1 = _host_prep(inputs)
    res = None
    for attempt in range(3):
        try:
            r = get_runner()
            r.place(in_maps)
            res = r.results(r.run())
            break
        except Exception:
            # transient NRT device errors have been observed; rebuild once
            _RUNNER = None
            if attempt == 2:
                raise
            import time
            time.sleep(10 + 20 * attempt)
    out = np.empty((B, HORIZONS), np.float32)
    for c in range(NCORES):
        out[c * BL:(c + 1) * BL] = res[c]["out"].T + b1
    return out

